# revision 4
# baseline (speedup 1.0000x reference)
"""Committee-vote histogram kernel for TRN2 (8 NeuronCores, data-parallel).

votes[b, c] = sum_m 1[argmax_c' (x[b] @ W[m, :, c'] + b[m, c']) == c]

Strategy per core (batch shard of 8192 rows):
  - x is decomposed host-side into an exact fp16 pair (x = xh + xl with
    residual ~2^-22|x|); likewise W and the bias. Logits are computed as
    xh@Wh + xh@Wl + xl@Wh (+bias), whose decomposition error (~2e-7) is at
    fp32 rounding level — validated exact-match against the fp32 reference.
  - The halves are stored host-side in [d', k, b] layout so loads are plain
    dense DMAs; 16 chunks of 512 rows stream on the two HWDGE rings
    (sync: xh, scalar: xl) with a 4-chunk prefetch window, keeping both
    rings saturated from the start while the whole shard stays resident.
  - Bias is added by seeding each PSUM accumulation group with a K=2
    matmul of ones against the replicated (bh|bl) rows.
  - Votes: the ACT engine drains each PSUM logit batch to SBUF fp32; DVE
    then runs reduce_max / is_ge / member-sum entirely on SBUF operands
    (dual-port mode), with the mask and counts in fp16 (exact: 0/1 masks,
    counts <= 8). Results stage contiguously in fp16; the host unscrambles.
"""

import os
import sys

import numpy as np

if os.path.isdir("/opt/trn_rl_repo") and "/opt/trn_rl_repo" not in sys.path:
    sys.path.insert(0, "/opt/trn_rl_repo")

import concourse.bass as bass
import concourse.tile as tile
from concourse import bacc, mybir
from concourse.bass import ts

F32 = mybir.dt.float32
F16 = mybir.dt.float16

B_FULL = 65536
D = 256
C = 10
M = 8
N_CORES = 8
B_SHARD = B_FULL // N_CORES  # 8192
P = 128

MC = M * C  # 80 logit columns per sample
CHUNK = 512  # batch rows per chunk = one vote batch (4 tiles of 128)
PREFETCH = 4  # chunks of load-DMA dispatched ahead of the compute loop


def build_nc(b_shard: int = B_SHARD) -> bass.Bass:
    chunk = min(CHUNK, b_shard)
    n_chunks = b_shard // chunk
    assert b_shard % chunk == 0
    tiles_per_chunk = chunk // P  # 4; one vote batch per chunk
    assert tiles_per_chunk == 4

    nc = bacc.Bacc("TRN2", target_bir_lowering=False)
    # x halves in [d', k, b] layout (d = 128k + d'), prepared host-side
    xh = nc.dram_tensor("xh", [P, 2, b_shard], F16, kind="ExternalInput")
    xl = nc.dram_tensor("xl", [P, 2, b_shard], F16, kind="ExternalInput")
    wh = nc.dram_tensor("wh", [D, MC], F16, kind="ExternalInput")
    wl = nc.dram_tensor("wl", [D, MC], F16, kind="ExternalInput")
    bc4 = nc.dram_tensor("bc4", [2, 4 * MC], F16, kind="ExternalInput")
    # per-chunk staging: ys[g, p, t*C + c] = votes[g*chunk + t*128 + p, c]
    ys = nc.dram_tensor("ys", [n_chunks, P, tiles_per_chunk * C], F16,
                        kind="ExternalOutput")

    with tile.TileContext(nc) as tc:
        with (
            tc.tile_pool(name="consts", bufs=1) as consts,
            tc.tile_pool(name="xt", bufs=1) as xt_pool,
            tc.tile_pool(name="lg", bufs=6, space="PSUM") as lg_pool,
            tc.tile_pool(name="lgs", bufs=3) as lgs_pool,
            tc.tile_pool(name="mx", bufs=3) as mx_pool,
            tc.tile_pool(name="eq", bufs=3) as eq_pool,
            tc.tile_pool(name="stg", bufs=3) as stg_pool,
        ):
            # W halves as [128 d', k, 80] where d = 128k + d'
            wh_sb = consts.tile([P, 2, MC], F16)
            nc.sync.dma_start(wh_sb, wh.rearrange("(k p) c -> p k c", p=P))
            wl_sb = consts.tile([P, 2, MC], F16)
            nc.scalar.dma_start(wl_sb, wl.rearrange("(k p) c -> p k c", p=P))
            bc4_sb = consts.tile([2, 4 * MC], F16)
            nc.scalar.dma_start(bc4_sb, bc4[:])
            ones2 = consts.tile([2, P], F16)
            nc.vector.memset(ones2, 1.0)

            # slot layout per chunk: [128 d', (hl k), chunk b]
            # slot 0: xh k=0, 1: xh k=1, 2: xl k=0, 3: xl k=1
            xt = [
                xt_pool.tile([P, 4, chunk], F16, name=f"xt{g}")
                for g in range(n_chunks)
            ]

            def load_chunk(g: int):
                sl = np.s_[:, :, g * chunk : (g + 1) * chunk]
                nc.sync.dma_start(xt[g][:, 0:2, :], xh[sl])
                nc.scalar.dma_start(xt[g][:, 2:4, :], xl[sl])

            for g in range(PREFETCH):
                load_chunk(g)

            for g in range(n_chunks):
                if g + PREFETCH < n_chunks:
                    load_chunk(g + PREFETCH)

                # logits for the chunk's 4 tiles, bias-seeded
                lg = lg_pool.tile([P, 4 * MC], F32)
                nc.tensor.matmul(
                    lg, lhsT=ones2, rhs=bc4_sb, start=True, stop=False
                )
                for j in range(4):
                    for k in range(2):
                        xh_c = xt[g][:, k, ts(j, P)]
                        xl_c = xt[g][:, 2 + k, ts(j, P)]
                        o = lg[:, ts(j, MC)]
                        nc.tensor.matmul(
                            o, lhsT=xh_c, rhs=wh_sb[:, k, :],
                            start=False, stop=False,
                        )
                        nc.tensor.matmul(
                            o, lhsT=xh_c, rhs=wl_sb[:, k, :],
                            start=False, stop=False,
                        )
                        nc.tensor.matmul(
                            o, lhsT=xl_c, rhs=wh_sb[:, k, :],
                            start=False, stop=(j == 3 and k == 1),
                        )

                # drain PSUM -> SBUF on the (otherwise idle) ACT engine so
                # the DVE vote ops run in dual-port all-SBUF mode
                lgs = lgs_pool.tile([P, 4 * MC], F32)
                nc.scalar.copy(lgs, lg[:])

                mx = mx_pool.tile([P, 4 * M], F32)
                nc.vector.reduce_max(
                    mx,
                    lgs[:].rearrange("p (a c) -> p a c", c=C),
                    axis=mybir.AxisListType.X,
                )
                # mask written (t, c, m)-ordered so the member-sum below
                # reduces over a unit-stride axis; fp16 mask is exact 0/1
                eq = eq_pool.tile([P, 4 * MC], F16)
                nc.vector.tensor_tensor(
                    out=eq[:].rearrange("p (t c m) -> p t m c", t=4, m=M, c=C),
                    in0=lgs[:].rearrange("p (t m c) -> p t m c", t=4, m=M, c=C),
                    in1=mx[:, :, None]
                    .rearrange("p (t m) c -> p t m c", t=4)
                    .broadcast_to([P, 4, M, C]),
                    op=mybir.AluOpType.is_ge,
                )
                stg = stg_pool.tile([P, 4 * C], F16)
                with nc.allow_low_precision("votes <= 8 are exact in fp16"):
                    nc.vector.reduce_sum(
                        stg[:],
                        eq[:].rearrange("p (t c m) -> p t c m", t=4, m=M, c=C),
                        axis=mybir.AxisListType.X,
                    )
                nc.sync.dma_start(ys[g], stg[:])
    nc.compile()
    return nc


_NC_CACHE: dict[int, bass.Bass] = {}


def _get_nc(b_shard: int) -> bass.Bass:
    if b_shard not in _NC_CACHE:
        _NC_CACHE[b_shard] = build_nc(b_shard)
    return _NC_CACHE[b_shard]


def _prep_inputs(x: np.ndarray, W: np.ndarray, b: np.ndarray):
    xf = np.asarray(x, dtype=np.float32)
    xh = xf.astype(np.float16)
    xl = (xf - xh.astype(np.float32)).astype(np.float16)
    # [B, 256] -> [128 d', 2 k, B] with d = 128k + d'
    parts = {
        "xh": xh.T.reshape(2, P, B_FULL).transpose(1, 0, 2),
        "xl": xl.T.reshape(2, P, B_FULL).transpose(1, 0, 2),
    }
    # m-major columns: col index = 10*m + c
    wf = np.asarray(W, dtype=np.float32).transpose(1, 0, 2).reshape(D, MC)
    whf = wf.astype(np.float16)
    wlf = (wf - whf.astype(np.float32)).astype(np.float16)
    bf = np.asarray(b, dtype=np.float32).reshape(MC)
    bh = bf.astype(np.float16)
    bl = (bf - bh.astype(np.float32)).astype(np.float16)
    bc4 = np.ascontiguousarray(
        np.stack([np.tile(bh, 4), np.tile(bl, 4)], axis=0)
    ).astype(np.float16)
    return parts, np.ascontiguousarray(whf), np.ascontiguousarray(wlf), bc4


def _unscramble(ys: np.ndarray) -> np.ndarray:
    # ys[g, p, t*C + c] -> votes[g*512 + t*128 + p, c]
    n_chunks = ys.shape[0]
    return (
        ys.reshape(n_chunks, P, 4, C)
        .transpose(0, 2, 1, 3)
        .reshape(n_chunks * 4 * P, C)
        .astype(np.float32)
    )


def kernel(x: np.ndarray, W: np.ndarray, b: np.ndarray, **_) -> np.ndarray:
    from concourse.bass_utils import run_bass_kernel_spmd

    assert x.shape == (B_FULL, D), x.shape
    parts, whf, wlf, bc4 = _prep_inputs(x, W, b)

    nc = _get_nc(B_SHARD)
    in_maps = [
        {
            **{
                k: np.ascontiguousarray(v[:, :, i * B_SHARD : (i + 1) * B_SHARD])
                for k, v in parts.items()
            },
            "wh": whf,
            "wl": wlf,
            "bc4": bc4,
        }
        for i in range(N_CORES)
    ]
    res = run_bass_kernel_spmd(nc, in_maps, core_ids=list(range(N_CORES)))
    return np.concatenate(
        [_unscramble(res.results[i]["ys"]) for i in range(N_CORES)], axis=0
    )


# revision 8
# speedup vs baseline: 1.1319x; 1.1319x over previous
"""Committee-vote histogram kernel for TRN2 (8 NeuronCores, data-parallel).

votes[b, c] = sum_m 1[argmax_c' (x[b] @ W[m, :, c'] + b[m, c']) == c]

Strategy per core (batch shard of 8192 rows):
  - x is decomposed host-side into an exact fp16 pair (x = xh + xl with
    residual ~2^-22|x|); likewise W and the bias. Logits are computed as
    xh@Wh + xh@Wl + xl@Wh (+bias), whose decomposition error (~2e-7) is at
    fp32 rounding level — validated exact-match against the fp32 reference.
  - The halves are stored host-side in [d', k, b] layout so loads are plain
    dense DMAs; 16 chunks of 512 rows stream on the two HWDGE rings
    (sync: xh, scalar: xl) with a 4-chunk prefetch window, keeping both
    rings saturated from the start while the whole shard stays resident.
  - Bias is added by seeding each PSUM accumulation group with a K=2
    matmul of ones against the replicated (bh|bl) rows.
  - Votes: the ACT engine drains each PSUM logit batch to SBUF fp32; DVE
    then runs reduce_max / is_ge / member-sum entirely on SBUF operands
    (dual-port mode), with the mask and counts in fp16 (exact: 0/1 masks,
    counts <= 8). Results stage contiguously in fp16; the host unscrambles.
"""

import os
import sys

import numpy as np

if os.path.isdir("/opt/trn_rl_repo") and "/opt/trn_rl_repo" not in sys.path:
    sys.path.insert(0, "/opt/trn_rl_repo")

import concourse.bass as bass
import concourse.tile as tile
from concourse import bacc, mybir
from concourse.bass import ts

F32 = mybir.dt.float32
F16 = mybir.dt.float16

B_FULL = 65536
D = 256
C = 10
M = 8
N_CORES = 8
B_SHARD = B_FULL // N_CORES  # 8192
P = 128

MC = M * C  # 80 logit columns per sample
CHUNK = 512  # batch rows per chunk = one vote batch (4 tiles of 128)
PREFETCH = 4  # chunks of load-DMA dispatched ahead of the compute loop


def build_nc(b_shard: int = B_SHARD) -> bass.Bass:
    chunk = min(CHUNK, b_shard)
    n_chunks = b_shard // chunk
    assert b_shard % chunk == 0
    tiles_per_chunk = chunk // P  # 4; one vote batch per chunk
    assert tiles_per_chunk == 4

    nc = bacc.Bacc("TRN2", target_bir_lowering=False)
    # x halves in [d', k, b] layout (d = 128k + d'), prepared host-side
    xh = nc.dram_tensor("xh", [P, 2, b_shard], F16, kind="ExternalInput")
    xl = nc.dram_tensor("xl", [P, 2, b_shard], F16, kind="ExternalInput")
    wh = nc.dram_tensor("wh", [D, MC], F16, kind="ExternalInput")
    wl = nc.dram_tensor("wl", [D, MC], F16, kind="ExternalInput")
    bc4 = nc.dram_tensor("bc4", [2, 4 * MC], F16, kind="ExternalInput")
    # per-chunk staging: ys[g, p, t*C + c] = votes[g*chunk + t*128 + p, c]
    ys = nc.dram_tensor("ys", [n_chunks, P, tiles_per_chunk * C], F32,
                        kind="ExternalOutput")

    with tile.TileContext(nc) as tc:
        with (
            tc.tile_pool(name="consts", bufs=1) as consts,
            tc.tile_pool(name="xt", bufs=1) as xt_pool,
            tc.tile_pool(name="lg", bufs=6, space="PSUM") as lg_pool,
            tc.tile_pool(name="lgs", bufs=3) as lgs_pool,
            tc.tile_pool(name="mx", bufs=3) as mx_pool,
            tc.tile_pool(name="eq", bufs=3) as eq_pool,
            tc.tile_pool(name="t4", bufs=3) as t4_pool,
            tc.tile_pool(name="t2", bufs=3) as t2_pool,
            tc.tile_pool(name="stg", bufs=3) as stg_pool,
        ):
            # W halves as [128 d', k, 80] where d = 128k + d'
            wh_sb = consts.tile([P, 2, MC], F16)
            nc.sync.dma_start(wh_sb, wh.rearrange("(k p) c -> p k c", p=P))
            wl_sb = consts.tile([P, 2, MC], F16)
            nc.scalar.dma_start(wl_sb, wl.rearrange("(k p) c -> p k c", p=P))
            bc4_sb = consts.tile([2, 4 * MC], F16)
            nc.scalar.dma_start(bc4_sb, bc4[:])
            ones2 = consts.tile([2, P], F16)
            nc.vector.memset(ones2, 1.0)

            # slot layout per chunk: [128 d', (hl k), chunk b]
            # slot 0: xh k=0, 1: xh k=1, 2: xl k=0, 3: xl k=1
            xt = [
                xt_pool.tile([P, 4, chunk], F16, name=f"xt{g}")
                for g in range(n_chunks)
            ]

            def load_chunk(g: int):
                sl = np.s_[:, :, g * chunk : (g + 1) * chunk]
                nc.sync.dma_start(xt[g][:, 0:2, :], xh[sl])
                nc.scalar.dma_start(xt[g][:, 2:4, :], xl[sl])

            for g in range(PREFETCH):
                load_chunk(g)

            for g in range(n_chunks):
                if g + PREFETCH < n_chunks:
                    load_chunk(g + PREFETCH)

                # logits for the chunk's 4 tiles, bias-seeded
                lg = lg_pool.tile([P, 4 * MC], F32)
                nc.tensor.matmul(
                    lg, lhsT=ones2, rhs=bc4_sb, start=True, stop=False
                )
                for j in range(4):
                    for k in range(2):
                        xh_c = xt[g][:, k, ts(j, P)]
                        xl_c = xt[g][:, 2 + k, ts(j, P)]
                        o = lg[:, ts(j, MC)]
                        nc.tensor.matmul(
                            o, lhsT=xh_c, rhs=wh_sb[:, k, :],
                            start=False, stop=False,
                        )
                        nc.tensor.matmul(
                            o, lhsT=xh_c, rhs=wl_sb[:, k, :],
                            start=False, stop=False,
                        )
                        nc.tensor.matmul(
                            o, lhsT=xl_c, rhs=wh_sb[:, k, :],
                            start=False, stop=(j == 3 and k == 1),
                        )

                # drain PSUM -> SBUF on the (otherwise idle) ACT engine so
                # the DVE vote ops run in dual-port all-SBUF mode
                lgs = lgs_pool.tile([P, 4 * MC], F32)
                nc.scalar.copy(lgs, lg[:])

                mx = mx_pool.tile([P, 4 * M], F32)
                nc.vector.reduce_max(
                    mx,
                    lgs[:].rearrange("p (a c) -> p a c", c=C),
                    axis=mybir.AxisListType.X,
                )
                # mask written (t, m, c)-ordered: every DVE operand below is
                # unit-stride/contiguous (strided DVE writes run ~3x slower)
                eq = eq_pool.tile([P, 4 * MC], F32)
                nc.vector.tensor_tensor(
                    out=eq[:].rearrange("p (t m c) -> p t m c", t=4, m=M, c=C),
                    in0=lgs[:].rearrange("p (t m c) -> p t m c", t=4, m=M, c=C),
                    in1=mx[:, :, None]
                    .rearrange("p (t m) c -> p t m c", t=4)
                    .broadcast_to([P, 4, M, C]),
                    op=mybir.AluOpType.is_ge,
                )
                # member-sum over m as a 3-level add tree: all reads/writes
                # stay contiguous (a strided reduce would run at ~1/3 speed)
                t4 = t4_pool.tile([P, 4 * 4 * C], F32)
                nc.vector.tensor_tensor(
                    out=t4[:].rearrange("p (t m c) -> p t m c", t=4, m=4, c=C),
                    in0=eq[:].rearrange("p (t m c) -> p t m c", t=4, m=M, c=C)[
                        :, :, 0:4, :
                    ],
                    in1=eq[:].rearrange("p (t m c) -> p t m c", t=4, m=M, c=C)[
                        :, :, 4:8, :
                    ],
                    op=mybir.AluOpType.add,
                )
                t2 = t2_pool.tile([P, 4 * 2 * C], F32)
                nc.vector.tensor_tensor(
                    out=t2[:].rearrange("p (t m c) -> p t m c", t=4, m=2, c=C),
                    in0=t4[:].rearrange("p (t m c) -> p t m c", t=4, m=4, c=C)[
                        :, :, 0:2, :
                    ],
                    in1=t4[:].rearrange("p (t m c) -> p t m c", t=4, m=4, c=C)[
                        :, :, 2:4, :
                    ],
                    op=mybir.AluOpType.add,
                )
                stg = stg_pool.tile([P, 4 * C], F32)
                nc.vector.tensor_tensor(
                    out=stg[:].rearrange("p (t c) -> p t c", t=4, c=C),
                    in0=t2[:].rearrange("p (t m c) -> p t m c", t=4, m=2, c=C)[
                        :, :, 0, :
                    ],
                    in1=t2[:].rearrange("p (t m c) -> p t m c", t=4, m=2, c=C)[
                        :, :, 1, :
                    ],
                    op=mybir.AluOpType.add,
                )
                nc.sync.dma_start(ys[g], stg[:])
    nc.compile()
    return nc


_NC_CACHE: dict[int, bass.Bass] = {}


def _get_nc(b_shard: int) -> bass.Bass:
    if b_shard not in _NC_CACHE:
        _NC_CACHE[b_shard] = build_nc(b_shard)
    return _NC_CACHE[b_shard]


def _prep_inputs(x: np.ndarray, W: np.ndarray, b: np.ndarray):
    xf = np.asarray(x, dtype=np.float32)
    xh = xf.astype(np.float16)
    xl = (xf - xh.astype(np.float32)).astype(np.float16)
    # [B, 256] -> [128 d', 2 k, B] with d = 128k + d'
    parts = {
        "xh": xh.T.reshape(2, P, B_FULL).transpose(1, 0, 2),
        "xl": xl.T.reshape(2, P, B_FULL).transpose(1, 0, 2),
    }
    # m-major columns: col index = 10*m + c
    wf = np.asarray(W, dtype=np.float32).transpose(1, 0, 2).reshape(D, MC)
    whf = wf.astype(np.float16)
    wlf = (wf - whf.astype(np.float32)).astype(np.float16)
    bf = np.asarray(b, dtype=np.float32).reshape(MC)
    bh = bf.astype(np.float16)
    bl = (bf - bh.astype(np.float32)).astype(np.float16)
    bc4 = np.ascontiguousarray(
        np.stack([np.tile(bh, 4), np.tile(bl, 4)], axis=0)
    ).astype(np.float16)
    return parts, np.ascontiguousarray(whf), np.ascontiguousarray(wlf), bc4


def _unscramble(ys: np.ndarray) -> np.ndarray:
    # ys[g, p, t*C + c] -> votes[g*512 + t*128 + p, c]
    n_chunks = ys.shape[0]
    return np.ascontiguousarray(
        ys.reshape(n_chunks, P, 4, C)
        .transpose(0, 2, 1, 3)
        .reshape(n_chunks * 4 * P, C),
        dtype=np.float32,
    )


def kernel(x: np.ndarray, W: np.ndarray, b: np.ndarray, **_) -> np.ndarray:
    from concourse.bass_utils import run_bass_kernel_spmd

    assert x.shape == (B_FULL, D), x.shape
    parts, whf, wlf, bc4 = _prep_inputs(x, W, b)

    nc = _get_nc(B_SHARD)
    in_maps = [
        {
            **{
                k: np.ascontiguousarray(v[:, :, i * B_SHARD : (i + 1) * B_SHARD])
                for k, v in parts.items()
            },
            "wh": whf,
            "wl": wlf,
            "bc4": bc4,
        }
        for i in range(N_CORES)
    ]
    res = run_bass_kernel_spmd(nc, in_maps, core_ids=list(range(N_CORES)))
    return np.concatenate(
        [_unscramble(res.results[i]["ys"]) for i in range(N_CORES)], axis=0
    )


# revision 14
# speedup vs baseline: 1.2129x; 1.0716x over previous
"""Committee-vote histogram kernel for TRN2 (8 NeuronCores, data-parallel).

votes[b, c] = sum_m 1[argmax_c' (x[b] @ W[m, :, c'] + b[m, c']) == c]

Strategy per core (batch shard of 8192 rows):
  - x is decomposed host-side into an exact fp16 pair (x = xh + xl with
    residual ~2^-22|x|); likewise W and the bias. Logits are computed as
    xh@Wh + xh@Wl + xl@Wh (+bias), whose decomposition error (~2e-7) is at
    fp32 rounding level — validated exact-match against the fp32 reference.
  - The halves are stored host-side in [d', k, b] layout so loads are plain
    dense DMAs; 16 chunks of 512 rows stream on the two HWDGE rings
    (sync: xh, scalar: xl) with a 4-chunk prefetch window, keeping both
    rings saturated from the start while the whole shard stays resident.
  - Bias is added by seeding each PSUM accumulation group with a K=2
    matmul of ones against the replicated (bh|bl) rows.
  - Votes: the ACT engine drains each PSUM logit batch to SBUF fp32; DVE
    then runs reduce_max / is_ge / member-sum entirely on SBUF operands
    (dual-port mode), with the mask and counts in fp16 (exact: 0/1 masks,
    counts <= 8). Results stage contiguously in fp16; the host unscrambles.
"""

import os
import sys

import numpy as np

if os.path.isdir("/opt/trn_rl_repo") and "/opt/trn_rl_repo" not in sys.path:
    sys.path.insert(0, "/opt/trn_rl_repo")

import concourse.bass as bass
import concourse.tile as tile
from concourse import bacc, mybir
from concourse.bass import ts

F32 = mybir.dt.float32
F16 = mybir.dt.float16
BF16 = mybir.dt.bfloat16

B_FULL = 65536
D = 256
C = 10
M = 8
N_CORES = 8
B_SHARD = B_FULL // N_CORES  # 8192
P = 128

MC = M * C  # 80 logit columns per sample
CHUNK = 512  # batch rows per chunk = one vote batch (4 tiles of 128)
PREFETCH = 4  # chunks of load-DMA dispatched ahead of the compute loop


def build_nc(b_shard: int = B_SHARD) -> bass.Bass:
    chunk = min(CHUNK, b_shard)
    n_chunks = b_shard // chunk
    assert b_shard % chunk == 0
    tiles_per_chunk = chunk // P  # 4; one vote batch per chunk
    assert tiles_per_chunk == 4

    nc = bacc.Bacc("TRN2", target_bir_lowering=False)
    # x halves in [d', k, b] layout (d = 128k + d'), prepared host-side
    xh = nc.dram_tensor("xh", [P, 2, b_shard], F16, kind="ExternalInput")
    xl = nc.dram_tensor("xl", [P, 2, b_shard], F16, kind="ExternalInput")
    wh = nc.dram_tensor("wh", [D, MC], F16, kind="ExternalInput")
    wl = nc.dram_tensor("wl", [D, MC], F16, kind="ExternalInput")
    bc4 = nc.dram_tensor("bc4", [2, 4 * MC], F16, kind="ExternalInput")
    # staging: ys[p, g*40 + t*C + c] = votes[g*chunk + t*128 + p, c]
    ys = nc.dram_tensor("ys", [P, n_chunks * tiles_per_chunk * C], BF16,
                        kind="ExternalOutput")

    with tile.TileContext(nc) as tc:
        with (
            tc.tile_pool(name="consts", bufs=1) as consts,
            tc.tile_pool(name="xt", bufs=1) as xt_pool,
            tc.tile_pool(name="lg", bufs=6, space="PSUM") as lg_pool,
            tc.tile_pool(name="mx", bufs=3) as mx_pool,
            tc.tile_pool(name="eq", bufs=3) as eq_pool,
            tc.tile_pool(name="t4", bufs=3) as t4_pool,
            tc.tile_pool(name="t2", bufs=3) as t2_pool,
            tc.tile_pool(name="stg", bufs=1) as stg_pool,
        ):
            # W halves as [128 d', k, 80] where d = 128k + d'
            wh_sb = consts.tile([P, 2, MC], F16)
            nc.sync.dma_start(wh_sb, wh.rearrange("(k p) c -> p k c", p=P))
            wl_sb = consts.tile([P, 2, MC], F16)
            nc.scalar.dma_start(wl_sb, wl.rearrange("(k p) c -> p k c", p=P))
            bc4_sb = consts.tile([2, 4 * MC], F16)
            nc.scalar.dma_start(bc4_sb, bc4[:])
            ones2 = consts.tile([2, P], F16)
            nc.vector.memset(ones2, 1.0)

            # slot layout per chunk: [128 d', (hl k), chunk b]
            # slot 0: xh k=0, 1: xh k=1, 2: xl k=0, 3: xl k=1
            xt = [
                xt_pool.tile([P, 4, chunk], F16, name=f"xt{g}")
                for g in range(n_chunks)
            ]
            # all load DMAs dispatched upfront: each HWDGE ring carries only
            # loads, so ring-space waits at the queue head block nothing
            for g in range(n_chunks):
                sl = np.s_[:, :, g * chunk : (g + 1) * chunk]
                nc.sync.dma_start(xt[g][:, 0:2, :], xh[sl])
                nc.scalar.dma_start(xt[g][:, 2:4, :], xl[sl])

            # votes accumulate here; stored once at the end
            stg = stg_pool.tile([P, n_chunks * 4 * C], BF16)

            for g in range(n_chunks):
                # logits for the chunk's 4 tiles, bias-seeded
                lg = lg_pool.tile([P, 4 * MC], F32)
                nc.tensor.matmul(
                    lg, lhsT=ones2, rhs=bc4_sb, start=True, stop=False
                )
                for j in range(4):
                    for k in range(2):
                        xh_c = xt[g][:, k, ts(j, P)]
                        xl_c = xt[g][:, 2 + k, ts(j, P)]
                        o = lg[:, ts(j, MC)]
                        nc.tensor.matmul(
                            o, lhsT=xh_c, rhs=wh_sb[:, k, :],
                            start=False, stop=False,
                        )
                        nc.tensor.matmul(
                            o, lhsT=xh_c, rhs=wl_sb[:, k, :],
                            start=False, stop=False,
                        )
                        nc.tensor.matmul(
                            o, lhsT=xl_c, rhs=wh_sb[:, k, :],
                            start=False, stop=(j == 3 and k == 1),
                        )

                mx = mx_pool.tile([P, 4 * M], F32)
                nc.vector.reduce_max(
                    mx,
                    lg[:].rearrange("p (a c) -> p a c", c=C),
                    axis=mybir.AxisListType.X,
                )
                # mask written (t, m, c)-ordered: every DVE operand below is
                # unit-stride/contiguous (strided DVE writes run ~3x slower);
                # bf16 mask/counts are exact (0/1 and sums <= 8)
                eq = eq_pool.tile([P, 4 * MC], BF16)
                nc.vector.tensor_tensor(
                    out=eq[:].rearrange("p (t m c) -> p t m c", t=4, m=M, c=C),
                    in0=lg[:].rearrange("p (t m c) -> p t m c", t=4, m=M, c=C),
                    in1=mx[:, :, None]
                    .rearrange("p (t m) c -> p t m c", t=4)
                    .broadcast_to([P, 4, M, C]),
                    op=mybir.AluOpType.is_ge,
                )
                # member-sum over m as a 3-level add tree: all-bf16 contiguous
                # tensor_tensor runs in the 2x DVE mode
                t4 = t4_pool.tile([P, 4 * 4 * C], BF16)
                nc.vector.tensor_tensor(
                    out=t4[:].rearrange("p (t m c) -> p t m c", t=4, m=4, c=C),
                    in0=eq[:].rearrange("p (t m c) -> p t m c", t=4, m=M, c=C)[
                        :, :, 0:4, :
                    ],
                    in1=eq[:].rearrange("p (t m c) -> p t m c", t=4, m=M, c=C)[
                        :, :, 4:8, :
                    ],
                    op=mybir.AluOpType.add,
                )
                t2 = t2_pool.tile([P, 4 * 2 * C], BF16)
                nc.vector.tensor_tensor(
                    out=t2[:].rearrange("p (t m c) -> p t m c", t=4, m=2, c=C),
                    in0=t4[:].rearrange("p (t m c) -> p t m c", t=4, m=4, c=C)[
                        :, :, 0:2, :
                    ],
                    in1=t4[:].rearrange("p (t m c) -> p t m c", t=4, m=4, c=C)[
                        :, :, 2:4, :
                    ],
                    op=mybir.AluOpType.add,
                )
                nc.vector.tensor_tensor(
                    out=stg[:, ts(g, 4 * C)].rearrange("p (t c) -> p t c", c=C),
                    in0=t2[:].rearrange("p (t m c) -> p t m c", t=4, m=2, c=C)[
                        :, :, 0, :
                    ],
                    in1=t2[:].rearrange("p (t m c) -> p t m c", t=4, m=2, c=C)[
                        :, :, 1, :
                    ],
                    op=mybir.AluOpType.add,
                )
            # single store at the end: the sync ring is idle once loads drain
            nc.sync.dma_start(ys[:], stg[:])
    nc.compile()
    return nc


_NC_CACHE: dict[int, bass.Bass] = {}


def _get_nc(b_shard: int) -> bass.Bass:
    if b_shard not in _NC_CACHE:
        _NC_CACHE[b_shard] = build_nc(b_shard)
    return _NC_CACHE[b_shard]


def _prep_inputs(x: np.ndarray, W: np.ndarray, b: np.ndarray):
    xf = np.asarray(x, dtype=np.float32)
    xh = xf.astype(np.float16)
    xl = (xf - xh.astype(np.float32)).astype(np.float16)
    # [B, 256] -> [128 d', 2 k, B] with d = 128k + d'
    parts = {
        "xh": xh.T.reshape(2, P, B_FULL).transpose(1, 0, 2),
        "xl": xl.T.reshape(2, P, B_FULL).transpose(1, 0, 2),
    }
    # m-major columns: col index = 10*m + c
    wf = np.asarray(W, dtype=np.float32).transpose(1, 0, 2).reshape(D, MC)
    whf = wf.astype(np.float16)
    wlf = (wf - whf.astype(np.float32)).astype(np.float16)
    bf = np.asarray(b, dtype=np.float32).reshape(MC)
    bh = bf.astype(np.float16)
    bl = (bf - bh.astype(np.float32)).astype(np.float16)
    bc4 = np.ascontiguousarray(
        np.stack([np.tile(bh, 4), np.tile(bl, 4)], axis=0)
    ).astype(np.float16)
    return parts, np.ascontiguousarray(whf), np.ascontiguousarray(wlf), bc4


def _unscramble(ys: np.ndarray) -> np.ndarray:
    # ys[p, g*40 + t*C + c] -> votes[g*512 + t*128 + p, c]
    n_chunks = ys.shape[1] // (4 * C)
    return np.ascontiguousarray(
        ys.astype(np.float32)
        .reshape(P, n_chunks, 4, C)
        .transpose(1, 2, 0, 3)
        .reshape(n_chunks * 4 * P, C)
    )


def kernel(x: np.ndarray, W: np.ndarray, b: np.ndarray, **_) -> np.ndarray:
    from concourse.bass_utils import run_bass_kernel_spmd

    assert x.shape == (B_FULL, D), x.shape
    parts, whf, wlf, bc4 = _prep_inputs(x, W, b)

    nc = _get_nc(B_SHARD)
    in_maps = [
        {
            **{
                k: np.ascontiguousarray(v[:, :, i * B_SHARD : (i + 1) * B_SHARD])
                for k, v in parts.items()
            },
            "wh": whf,
            "wl": wlf,
            "bc4": bc4,
        }
        for i in range(N_CORES)
    ]
    res = run_bass_kernel_spmd(nc, in_maps, core_ids=list(range(N_CORES)))
    return np.concatenate(
        [_unscramble(res.results[i]["ys"]) for i in range(N_CORES)], axis=0
    )
